# revision 18
# baseline (speedup 1.0000x reference)
"""ALSHConv Trainium2 kernel (8 NeuronCores, SPMD).

Device: the memory-bound vote convolution, decomposed as a single
K=64-channel matmul producing 9 per-tap partial planes S[t, x] =
sum_c input[c, x] * a[c*9+t] (float32r TensorEngine, PSUM-packed 4x32
partitions x 4 banks, DVE-evacuated, DMA'd out).

Host: 9-tap shifted combine (+ analytic constant-channel map + bias),
vote binning/histogram, hash-table init over the 256 kernels, argmax,
active-set gather — all tiny next to the 268MB input read.

Sharding: data-parallel over batch — 2 images per core; kernel bank,
hash weights replicated; per-core partial S gathered on host.
"""

import sys
import types

import numpy as np

N_CORES = 8
NIMG, C, H, W = 16, 64, 256, 256
O, KK, M_TERMS, T, R = 256, 3, 9, 8192, 4.0
IMG_PER_CORE = NIMG // N_CORES
ROWS_PER_CHUNK = 128  # rows per pair-chunk (2 blocks of 64 rows on 128 parts)
CHUNKS_PER_IMG = H // ROWS_PER_CHUNK
N_GRP = IMG_PER_CORE * CHUNKS_PER_IMG  # 16 groups per core
GRP_X = ROWS_PER_CHUNK * W  # 8192 x-positions per group
MM_N = 512
MM_PER_GRP = GRP_X // MM_N  # 16


def _install_ntff_shim():
    if "antenv.axon_hooks" in sys.modules:
        return
    try:
        import antenv
        from trn_agent_boot.trn_boot import _ntff_profile_via_ctypes
    except ImportError:
        return
    mod = types.ModuleType("antenv.axon_hooks")
    _hook = [_ntff_profile_via_ctypes("/opt/axon/libaxon_pjrt.so")]
    mod.get_axon_ntff_profile_hook = lambda: _hook[0]
    mod.set_axon_ntff_profile_hook = lambda h: _hook.__setitem__(0, h)
    sys.modules["antenv.axon_hooks"] = mod
    antenv.axon_hooks = mod


def _install_drain_split_patch():
    import concourse.tile as tile
    from concourse import mybir
    from concourse.vector_clock import ScopedClock

    if getattr(tile.TileContext, "_drain_split_patched", False):
        return

    def _drain_and_barrier(self, tick_clock, wait_clock):
        nop = self.nc.sync.nop(nofuse=True, hint="drain_waits")
        wait_clock.add_sem_waits(nop.ins, ScopedClock({None: tick_clock.global_clock}))
        si = nop.ins.sync_info
        waits = list(si.on_wait) if si and si.on_wait else []
        if len(waits) > 1:
            si.on_wait[:] = waits[:1]
            for w in waits[1:]:
                extra = self.nc.sync.nop(nofuse=True, hint="drain_waits")
                esi = extra.ins.sync_info
                if esi is None:
                    esi = mybir.SyncInfo(on_wait=[], on_update=[])
                    extra.ins.sync_info = esi
                esi.on_wait.append(w)
        self.nc.sync.drain()
        self.nc.all_engine_barrier()
        assert self.sems is not None
        popped = self.nc._tile_sem_poison_stack.pop()
        assert popped is self._sem_poison
        self.nc.clear_and_free_semaphores(list(self.sems.allocated().values()))
        self.nc.all_engine_barrier()

    tile.TileContext._drain_and_barrier = _drain_and_barrier
    tile.TileContext._drain_split_patched = True


def _split_multi_waits(nc, limit=1):
    """walrus here encodes at most one sync wait per instruction; move
    surplus waits onto preceding single-wait NoOps on the same engine."""
    from concourse import mybir

    uid = 0
    for f in nc.m.functions:
        for bb in f.blocks:
            out = []
            changed = False
            for inst in bb.instructions:
                si = inst.sync_info
                waits = list(si.on_wait) if si is not None and si.on_wait else []
                if len(waits) > limit:
                    for w in waits[:-limit]:
                        uid += 1
                        nop = mybir.InstNoOp(name=f"ant-splitw-{uid}", engine=inst.engine)
                        nop.sync_info = mybir.SyncInfo(on_wait=[w], on_update=[])
                        out.append(nop)
                    si.on_wait[:] = waits[-limit:]
                    changed = True
                out.append(inst)
            if changed:
                bb.instructions[:] = out
    return nc


_NC_CACHE = {}


def _build_nc():
    import concourse.bass as bass
    import concourse.tile as tile
    from concourse import mybir

    _install_drain_split_patch()
    f32 = mybir.dt.float32
    f16 = mybir.dt.float16

    nc = bass.Bass(target_bir_lowering=False)
    inp_d = nc.declare_dram_parameter(
        "input", [IMG_PER_CORE, C, H, W], f16, isOutput=False
    )
    w_d = nc.declare_dram_parameter("w", [128, 32], f16, isOutput=False)
    # sout[tg, j, t, q*2048 + g*512 + n] = S[t, (4*tg+q)*8192 + (4g+j)*512 + n]
    sout_d = nc.declare_dram_parameter(
        "sout", [N_GRP, 4, 9, 16 * MM_N], f16, isOutput=True
    )

    with tile.TileContext(nc) as tc:
        with (
            tc.tile_pool(name="wpool", bufs=1) as wpool,
            tc.tile_pool(name="inpool", bufs=3) as inpool,
            tc.tile_pool(name="pspool", bufs=2, space="PSUM") as pspool,
            tc.tile_pool(name="evpool", bufs=4) as evpool,
        ):
            wt = wpool.tile([128, 32], f16)
            nc.sync.dma_start(out=wt[:], in_=w_d[:])
            _EV = [None]

            for grp in range(N_GRP):
                n, p = divmod(grp, CHUNKS_PER_IMG)
                # pair-chunk: 64 image rows as 2 blocks of 32 rows on
                # partitions (b*64 + c), free = (y_local, x)
                src = inp_d[n, :, p * ROWS_PER_CHUNK : (p + 1) * ROWS_PER_CHUNK, :]
                src = src.rearrange("c (b y) x -> b c (y x)", b=2)
                it = inpool.tile([128, GRP_X // 2], f16)
                # split across both HWDGE rings for DMA parallelism
                nc.sync.dma_start(out=it[0:64], in_=src[0:1])
                nc.scalar.dma_start(out=it[64:128], in_=src[1:2])

                for quarter in range(4):
                    b = quarter // 2
                    tile_idx = 4 * grp + quarter
                    q = tile_idx % 4
                    if q == 0:
                        ev = evpool.tile([128, 16 * MM_N], f16)
                        _EV[0] = ev
                    ev = _EV[0]
                    ps = pspool.tile([128, 4 * MM_N], f32)
                    for m16 in range(16):
                        m32 = (quarter % 2) * 16 + m16
                        j, g = m16 % 4, m16 // 4
                        nc.tensor.matmul(
                            ps[32 * j : 32 * j + 32, MM_N * g : MM_N * (g + 1)],
                            wt[64 * b : 64 * b + 64, :],
                            it[64 * b : 64 * b + 64, MM_N * m32 : MM_N * (m32 + 1)],
                            start=True,
                            stop=True,
                            tile_position=(64 * b, 32 * j),
                        )
                    evq = ev[:, q * 4 * MM_N : (q + 1) * 4 * MM_N]
                    if quarter % 2 == 0:
                        nc.vector.tensor_copy(evq, ps[:])
                    else:
                        nc.scalar.copy(evq, ps[:])
                    if q == 3:
                        for j in range(4):
                            nc.gpsimd.dma_start(
                                out=sout_d[tile_idx // 4, j],
                                in_=ev[32 * j : 32 * j + 9, :],
                            )
    return nc


def _get_nc():
    if "nc" not in _NC_CACHE:
        nc = _build_nc()
        _split_multi_waits(nc)
        _NC_CACHE["nc"] = nc
    return _NC_CACHE["nc"]


def _run_device(input_arr, w128, trace=False):
    from concourse.bass_utils import run_bass_kernel_spmd

    if trace:
        _install_ntff_shim()
    nc = _get_nc()
    in_maps = []
    for core in range(N_CORES):
        shard = np.ascontiguousarray(
            input_arr[core * IMG_PER_CORE : (core + 1) * IMG_PER_CORE]
        )
        in_maps.append({"input": shard, "w": w128})
    res = run_bass_kernel_spmd(
        nc, in_maps, core_ids=list(range(N_CORES)), trace=trace
    )
    return res


def _decode_s(results):
    """sout[tg, j, t, q*2048+g*512+n] = S[t, (4tg+q)*8192 + (4g+j)*512 + n]."""
    S = np.empty((NIMG, 9, H * W), np.float32)
    tiles_per_img = H * W // (16 * MM_N)
    for core in range(N_CORES):
        sout = results[core]["sout"].astype(np.float32)  # [4, 4, 9, 8192]
        v = sout.reshape(-1, 4, 9, 4, 4, MM_N)  # [tg, j, t, q, g, n]
        v = v.transpose(0, 3, 4, 1, 2, 5)  # [tg, q, g, j, t, n]
        v = v.reshape(-1, 16, 9, MM_N)  # [tile=4tg+q, m16=4g+j, t, n]
        for li in range(IMG_PER_CORE):
            img = core * IMG_PER_CORE + li
            chunk = v[li * tiles_per_img : (li + 1) * tiles_per_img]
            S[img] = chunk.transpose(2, 0, 1, 3).reshape(9, H * W)
    return S.reshape(NIMG, 9, H, W)


def _shifted(img, dy, dx):
    p = np.pad(img, ((0, 0), (1, 1), (1, 1)))
    return p[:, 1 + dy : 1 + dy + H, 1 + dx : 1 + dx + W]


def _host_post(S, kernels, a, b):
    a = np.asarray(a, np.float32)
    b = np.asarray(b, np.float32)
    kernels = np.asarray(kernels, np.float32)
    aq = a[C * 9 :].reshape(3, 3)

    ones = np.ones((1, H, W), np.float32)
    qmap = np.zeros((1, H, W), np.float32)
    for ky in range(3):
        for kx in range(3):
            qmap += np.float32(0.5) * aq[ky, kx] * _shifted(ones, ky - 1, kx - 1)

    dotted = np.zeros((NIMG, H, W), np.float32)
    for ky in range(3):
        for kx in range(3):
            dotted += _shifted(S[:, ky * 3 + kx], ky - 1, kx - 1)
    dotted += qmap

    votes = np.floor((dotted.reshape(-1) + b[0]) / np.float32(R))
    vidx = np.abs(np.fmod(votes, T)).astype(np.int32)
    count = np.bincount(vidx, minlength=T).astype(np.float32)
    index = int(np.argmax(count))

    # ---- hash-table init over the kernel bank (mirrors reference) ----
    flat = kernels.reshape(O, -1)
    n2 = np.sum(flat * flat, axis=1, keepdims=True, dtype=np.float32)
    pows = np.concatenate([n2 ** (i + 1) for i in range(M_TERMS)], axis=1).astype(
        np.float32
    )
    P = np.concatenate([flat, pows], axis=1)
    hk = np.floor((P @ a + b[0]) / np.float32(R))
    idx = np.abs(np.fmod(hk, T)).astype(np.int32)
    onehot = np.zeros((O, T), np.int32)
    onehot[np.arange(O), idx] = 1
    excl = np.cumsum(onehot, axis=0) - onehot
    pos = excl[np.arange(O), idx]
    table = np.zeros((T, 2 * O), np.int32)
    table[idx, pos] = np.arange(O, dtype=np.int32)
    row_len = onehot.sum(axis=0)

    L = row_len[index]
    slot = np.arange(2 * O)
    rows = np.where(slot < L, table[index], O)
    rows = np.sort(rows)
    valid = (slot < L).astype(kernels.dtype)
    active = kernels[np.clip(rows, 0, O - 1)] * valid[:, None, None, None]
    return active, count, np.int32(index)


def kernel(input, kernels, a, b, _trace=False, _return_res=False):
    input = np.asarray(input, np.float32).astype(np.float16)
    a32 = np.asarray(a, np.float32)
    w9 = a32[: C * 9].reshape(C, 9)
    w128 = np.zeros((128, 32), np.float16)
    w128[0:C, 0:9] = w9.astype(np.float16)
    w128[C : 2 * C, 0:9] = w9.astype(np.float16)

    res = _run_device(input, w128, trace=_trace)
    S = _decode_s(res.results)
    out = _host_post(S, kernels, a, b)
    if _return_res:
        return out, res
    return out


# revision 19
# speedup vs baseline: 1.3399x; 1.3399x over previous
"""ALSHConv Trainium2 kernel (8 NeuronCores, SPMD).

Device: the memory-bound vote convolution, decomposed as a single
K=64-channel matmul producing 9 per-tap partial planes S[t, x] =
sum_c input[c, x] * a[c*9+t] (float32r TensorEngine, PSUM-packed 4x32
partitions x 4 banks, DVE-evacuated, DMA'd out).

Host: 9-tap shifted combine (+ analytic constant-channel map + bias),
vote binning/histogram, hash-table init over the 256 kernels, argmax,
active-set gather — all tiny next to the 268MB input read.

Sharding: data-parallel over batch — 2 images per core; kernel bank,
hash weights replicated; per-core partial S gathered on host.
"""

import sys
import types

import numpy as np

N_CORES = 8
NIMG, C, H, W = 16, 64, 256, 256
O, KK, M_TERMS, T, R = 256, 3, 9, 8192, 4.0
IMG_PER_CORE = NIMG // N_CORES
ROWS_PER_CHUNK = 64  # rows per pair-chunk (2 blocks of 32 rows on 128 parts)
CHUNKS_PER_IMG = H // ROWS_PER_CHUNK
N_GRP = IMG_PER_CORE * CHUNKS_PER_IMG  # 16 groups per core
GRP_X = ROWS_PER_CHUNK * W  # 8192 x-positions per group
MM_N = 512
MM_PER_GRP = GRP_X // MM_N  # 16


def _install_ntff_shim():
    if "antenv.axon_hooks" in sys.modules:
        return
    try:
        import antenv
        from trn_agent_boot.trn_boot import _ntff_profile_via_ctypes
    except ImportError:
        return
    mod = types.ModuleType("antenv.axon_hooks")
    _hook = [_ntff_profile_via_ctypes("/opt/axon/libaxon_pjrt.so")]
    mod.get_axon_ntff_profile_hook = lambda: _hook[0]
    mod.set_axon_ntff_profile_hook = lambda h: _hook.__setitem__(0, h)
    sys.modules["antenv.axon_hooks"] = mod
    antenv.axon_hooks = mod


def _install_drain_split_patch():
    import concourse.tile as tile
    from concourse import mybir
    from concourse.vector_clock import ScopedClock

    if getattr(tile.TileContext, "_drain_split_patched", False):
        return

    def _drain_and_barrier(self, tick_clock, wait_clock):
        nop = self.nc.sync.nop(nofuse=True, hint="drain_waits")
        wait_clock.add_sem_waits(nop.ins, ScopedClock({None: tick_clock.global_clock}))
        si = nop.ins.sync_info
        waits = list(si.on_wait) if si and si.on_wait else []
        if len(waits) > 1:
            si.on_wait[:] = waits[:1]
            for w in waits[1:]:
                extra = self.nc.sync.nop(nofuse=True, hint="drain_waits")
                esi = extra.ins.sync_info
                if esi is None:
                    esi = mybir.SyncInfo(on_wait=[], on_update=[])
                    extra.ins.sync_info = esi
                esi.on_wait.append(w)
        self.nc.sync.drain()
        self.nc.all_engine_barrier()
        assert self.sems is not None
        popped = self.nc._tile_sem_poison_stack.pop()
        assert popped is self._sem_poison
        self.nc.clear_and_free_semaphores(list(self.sems.allocated().values()))
        self.nc.all_engine_barrier()

    tile.TileContext._drain_and_barrier = _drain_and_barrier
    tile.TileContext._drain_split_patched = True


def _split_multi_waits(nc, limit=1):
    """walrus here encodes at most one sync wait per instruction; move
    surplus waits onto preceding single-wait NoOps on the same engine."""
    from concourse import mybir

    uid = 0
    for f in nc.m.functions:
        for bb in f.blocks:
            out = []
            changed = False
            for inst in bb.instructions:
                si = inst.sync_info
                waits = list(si.on_wait) if si is not None and si.on_wait else []
                if len(waits) > limit:
                    for w in waits[:-limit]:
                        uid += 1
                        nop = mybir.InstNoOp(name=f"ant-splitw-{uid}", engine=inst.engine)
                        nop.sync_info = mybir.SyncInfo(on_wait=[w], on_update=[])
                        out.append(nop)
                    si.on_wait[:] = waits[-limit:]
                    changed = True
                out.append(inst)
            if changed:
                bb.instructions[:] = out
    return nc


_NC_CACHE = {}


def _build_nc():
    import concourse.bass as bass
    import concourse.tile as tile
    from concourse import mybir

    _install_drain_split_patch()
    f32 = mybir.dt.float32
    f16 = mybir.dt.float16

    nc = bass.Bass(target_bir_lowering=False)
    inp_d = nc.declare_dram_parameter(
        "input", [IMG_PER_CORE, C, H, W], f16, isOutput=False
    )
    w_d = nc.declare_dram_parameter("w", [128, 32], f16, isOutput=False)
    # sout[tg, j, t, q*2048 + g*512 + n] = S[t, (4*tg+q)*8192 + (4g+j)*512 + n]
    sout_d = nc.declare_dram_parameter(
        "sout", [N_GRP // 2, 4, 9, 16 * MM_N], f16, isOutput=True
    )

    with tile.TileContext(nc) as tc:
        with (
            tc.tile_pool(name="wpool", bufs=1) as wpool,
            tc.tile_pool(name="inpool", bufs=4) as inpool,
            tc.tile_pool(name="pspool", bufs=2, space="PSUM") as pspool,
            tc.tile_pool(name="evpool", bufs=4) as evpool,
        ):
            wt = wpool.tile([128, 32], f16)
            nc.sync.dma_start(out=wt[:], in_=w_d[:])
            _EV = [None]

            for grp in range(N_GRP):
                n, p = divmod(grp, CHUNKS_PER_IMG)
                # pair-chunk: 64 image rows as 2 blocks of 32 rows on
                # partitions (b*64 + c), free = (y_local, x)
                src = inp_d[n, :, p * ROWS_PER_CHUNK : (p + 1) * ROWS_PER_CHUNK, :]
                src = src.rearrange("c (b y) x -> b c (y x)", b=2)
                it = inpool.tile([128, GRP_X // 2], f16)
                # split across both HWDGE rings for DMA parallelism
                nc.sync.dma_start(out=it[0:64], in_=src[0:1])
                nc.scalar.dma_start(out=it[64:128], in_=src[1:2])

                for half in range(2):
                    b = half
                    tile_idx = 2 * grp + half
                    q = tile_idx % 4
                    if q == 0:
                        ev = evpool.tile([128, 16 * MM_N], f16)
                        _EV[0] = ev
                    ev = _EV[0]
                    ps = pspool.tile([128, 4 * MM_N], f32)
                    for m16 in range(16):
                        j, g = m16 % 4, m16 // 4
                        nc.tensor.matmul(
                            ps[32 * j : 32 * j + 32, MM_N * g : MM_N * (g + 1)],
                            wt[64 * b : 64 * b + 64, :],
                            it[64 * b : 64 * b + 64, MM_N * m16 : MM_N * (m16 + 1)],
                            start=True,
                            stop=True,
                            tile_position=(64 * b, 32 * j),
                        )
                    evq = ev[:, q * 4 * MM_N : (q + 1) * 4 * MM_N]
                    if half == 0:
                        nc.vector.tensor_copy(evq, ps[:])
                    else:
                        nc.scalar.copy(evq, ps[:])
                    if q == 3:
                        for j in range(4):
                            nc.gpsimd.dma_start(
                                out=sout_d[tile_idx // 4, j],
                                in_=ev[32 * j : 32 * j + 9, :],
                            )
    return nc


def _get_nc():
    if "nc" not in _NC_CACHE:
        nc = _build_nc()
        _split_multi_waits(nc)
        _NC_CACHE["nc"] = nc
    return _NC_CACHE["nc"]


def _run_device(input_arr, w128, trace=False):
    from concourse.bass_utils import run_bass_kernel_spmd

    if trace:
        _install_ntff_shim()
    nc = _get_nc()
    in_maps = []
    for core in range(N_CORES):
        shard = np.ascontiguousarray(
            input_arr[core * IMG_PER_CORE : (core + 1) * IMG_PER_CORE]
        )
        in_maps.append({"input": shard, "w": w128})
    res = run_bass_kernel_spmd(
        nc, in_maps, core_ids=list(range(N_CORES)), trace=trace
    )
    return res


def _decode_s(results):
    """sout[tg, j, t, q*2048+g*512+n] = S[t, (4tg+q)*8192 + (4g+j)*512 + n]."""
    S = np.empty((NIMG, 9, H * W), np.float32)
    tiles_per_img = H * W // (16 * MM_N)
    for core in range(N_CORES):
        sout = results[core]["sout"].astype(np.float32)  # [4, 4, 9, 8192]
        v = sout.reshape(-1, 4, 9, 4, 4, MM_N)  # [tg, j, t, q, g, n]
        v = v.transpose(0, 3, 4, 1, 2, 5)  # [tg, q, g, j, t, n]
        v = v.reshape(-1, 16, 9, MM_N)  # [tile=4tg+q, m16=4g+j, t, n]
        for li in range(IMG_PER_CORE):
            img = core * IMG_PER_CORE + li
            chunk = v[li * tiles_per_img : (li + 1) * tiles_per_img]
            S[img] = chunk.transpose(2, 0, 1, 3).reshape(9, H * W)
    return S.reshape(NIMG, 9, H, W)


def _shifted(img, dy, dx):
    p = np.pad(img, ((0, 0), (1, 1), (1, 1)))
    return p[:, 1 + dy : 1 + dy + H, 1 + dx : 1 + dx + W]


def _host_post(S, kernels, a, b):
    a = np.asarray(a, np.float32)
    b = np.asarray(b, np.float32)
    kernels = np.asarray(kernels, np.float32)
    aq = a[C * 9 :].reshape(3, 3)

    ones = np.ones((1, H, W), np.float32)
    qmap = np.zeros((1, H, W), np.float32)
    for ky in range(3):
        for kx in range(3):
            qmap += np.float32(0.5) * aq[ky, kx] * _shifted(ones, ky - 1, kx - 1)

    dotted = np.zeros((NIMG, H, W), np.float32)
    for ky in range(3):
        for kx in range(3):
            dotted += _shifted(S[:, ky * 3 + kx], ky - 1, kx - 1)
    dotted += qmap

    votes = np.floor((dotted.reshape(-1) + b[0]) / np.float32(R))
    vidx = np.abs(np.fmod(votes, T)).astype(np.int32)
    count = np.bincount(vidx, minlength=T).astype(np.float32)
    index = int(np.argmax(count))

    # ---- hash-table init over the kernel bank (mirrors reference) ----
    flat = kernels.reshape(O, -1)
    n2 = np.sum(flat * flat, axis=1, keepdims=True, dtype=np.float32)
    pows = np.concatenate([n2 ** (i + 1) for i in range(M_TERMS)], axis=1).astype(
        np.float32
    )
    P = np.concatenate([flat, pows], axis=1)
    hk = np.floor((P @ a + b[0]) / np.float32(R))
    idx = np.abs(np.fmod(hk, T)).astype(np.int32)
    onehot = np.zeros((O, T), np.int32)
    onehot[np.arange(O), idx] = 1
    excl = np.cumsum(onehot, axis=0) - onehot
    pos = excl[np.arange(O), idx]
    table = np.zeros((T, 2 * O), np.int32)
    table[idx, pos] = np.arange(O, dtype=np.int32)
    row_len = onehot.sum(axis=0)

    L = row_len[index]
    slot = np.arange(2 * O)
    rows = np.where(slot < L, table[index], O)
    rows = np.sort(rows)
    valid = (slot < L).astype(kernels.dtype)
    active = kernels[np.clip(rows, 0, O - 1)] * valid[:, None, None, None]
    return active, count, np.int32(index)


def kernel(input, kernels, a, b, _trace=False, _return_res=False):
    input = np.asarray(input, np.float32).astype(np.float16)
    a32 = np.asarray(a, np.float32)
    w9 = a32[: C * 9].reshape(C, 9)
    w128 = np.zeros((128, 32), np.float16)
    w128[0:C, 0:9] = w9.astype(np.float16)
    w128[C : 2 * C, 0:9] = w9.astype(np.float16)

    res = _run_device(input, w128, trace=_trace)
    S = _decode_s(res.results)
    out = _host_post(S, kernels, a, b)
    if _return_res:
        return out, res
    return out
